# revision 6
# baseline (speedup 1.0000x reference)
"""Trainium2 Bass kernel for nn_MultiHeadAttention_88210038326473 (v3, fp8).

Reference computation (B=4, S=2048, HID=2048, H=16, DH=128):
    Q = queries @ Wq.T + bq ; K = keys @ Wk.T + bk ; V = keys @ Wv.T + bv
    per-head scores = Qh Kh^T / sqrt(HID), key-padding + causal mask,
    softmax, out = attn @ Vh, concat heads, + queries residual.

Sharding: 8 cores = 4 batches x 2 head-groups (8 heads each). Each core
computes attn_out[b, :, hg*1024:(hg+1)*1024] ([1024, 2048] bf16,
transposed); host transposes back, adds the queries residual in fp32,
and assembles.

v3 vs v2: fp8(e4m3) everywhere the error budget allows, validated by
host-side emulation at rel_err ~4e-3 (budget 2e-2; the fp32 queries
residual dominates the output norm, diluting attention error ~12x):
  - projection inputs + weights in fp8, contracted 2 f-tiles per
    DoubleRow matmul (2x PE throughput),
  - probabilities (Exp output) and V in fp8; the attn V-matmul and the
    row-sum matmul consume full k-tile PAIRS per DoubleRow matmul,
  - scores stay bf16 (contraction=128 cannot DoubleRow; fp8 wouldn't be
    faster and costs accuracy).
K^T/V/Q^T all SBUF-resident, paired Exp over 2-bank PSUM groups,
software-pipelined attention (scores one pair-group ahead of V/sum),
residual add on host.
"""

import math

import numpy as np

B, S, HID, H, DH = 4, 2048, 2048, 16, 128
NCORES = 8
HPC = 8          # heads per core
EH = HPC * DH    # 1024 e-dims per core
SCALE = 1.0 / math.sqrt(HID)
QC = 512         # attention q-chunk
NQC = S // QC    # 4
NKT = S // DH    # 16 k-tiles
NF = HID // DH   # 16 f-tiles (contraction)
PC = 512         # projection s-chunk (matmul moving N)
NPC = S // PC    # 4
FG = 4           # f-tiles per DMA group
NEG_BIAS = np.float32(-1.0e30)
USE_FP8 = True
COMPUTE_MAX_WAITS = 1


CTRL_OPS = ("InstDrain", "InstNoOp", "InstEventSemaphore", "InstISA")


def _split_excess_waits(nc, max_waits=1, compute_max_waits=None):
    """walrus in this container rejects >1 sem-wait per CTRL-class
    instruction. Move excess waits onto preceding NoOps on the same
    engine."""
    import concourse.mybir as mybir

    if compute_max_waits is None:
        compute_max_waits = max_waits
    n_split = 0
    for fn in nc.m.functions:
        for blk in fn.blocks:
            insts = list(blk.instructions)
            out = []
            changed = False
            for ins in insts:
                lim = (
                    max_waits
                    if type(ins).__name__ in CTRL_OPS
                    else compute_max_waits
                )
                si = ins.sync_info
                if si is not None and si.on_wait and len(si.on_wait) > lim:
                    waits = list(si.on_wait)
                    carriers, rest = waits[:-lim], waits[-lim:]
                    for i in range(0, len(carriers), max_waits):
                        chunk = carriers[i : i + max_waits]
                        out.append(
                            mybir.InstNoOp(
                                name=f"{ins.name}-ws{i}",
                                engine=ins.engine,
                                bass_nofuse=True,
                                sync_info=mybir.SyncInfo(on_wait=chunk, on_update=[]),
                            )
                        )
                        n_split += 1
                    ins.sync_info = mybir.SyncInfo(
                        on_wait=rest, on_update=list(si.on_update)
                    )
                    changed = True
                out.append(ins)
            if changed:
                blk.instructions = out
    return n_split


_CACHE = {}


def _build(fast=True, has_bias=False, reps=1, use_fp8=None, scale=None):
    """Build the (core-uniform) Bass program. Returns nc.

    fast=True drops the key-padding bias from the exp (valid when no key
    is padding -- checked on host). has_bias=False skips projection
    biases (valid when bq=bk=bv=0 -- checked on host). use_fp8 switches
    projection inputs/weights + probabilities/V to fp8e4 with DoubleRow
    matmuls. reps>1 repeats the whole body; scale={"k"/"q"/"attn": n}
    repeats a phase (timing instruments)."""
    scale = scale or {}
    if use_fp8 is None:
        use_fp8 = USE_FP8
    key = ("nc", fast, has_bias, reps, use_fp8, tuple(sorted(scale.items())))
    if key in _CACHE:
        return _CACHE[key]

    import concourse.bass as bass
    import concourse.mybir as mybir
    from concourse.tile import TileContext

    F32 = mybir.dt.float32
    F32R = mybir.dt.float32r
    ADT = mybir.dt.bfloat16
    IDT = mybir.dt.float8e4 if use_fp8 else ADT  # proj inputs + weights
    EDT = mybir.dt.float8e4 if use_fp8 else ADT  # probabilities + V
    EXP = mybir.ActivationFunctionType.Exp
    IDENT = mybir.ActivationFunctionType.Identity

    nc = bass.Bass("TRN2", target_bir_lowering=False, debug=False)

    qT = nc.dram_tensor("qT", [HID, S], IDT, kind="ExternalInput")
    kT = nc.dram_tensor("kT", [HID, S], IDT, kind="ExternalInput")
    wqT = nc.dram_tensor("wqT", [HID, EH], IDT, kind="ExternalInput")
    wkT = nc.dram_tensor("wkT", [HID, EH], IDT, kind="ExternalInput")
    wvT = nc.dram_tensor("wvT", [HID, EH], IDT, kind="ExternalInput")
    bq_d = nc.dram_tensor("bq_d", [DH, HPC], F32, kind="ExternalInput")
    bk_d = nc.dram_tensor("bk_d", [DH, HPC], F32, kind="ExternalInput")
    bv_d = nc.dram_tensor("bv_d", [1, EH], ADT, kind="ExternalInput")
    kbias_d = nc.dram_tensor("kbias_d", [DH, NKT], F32, kind="ExternalInput")
    tri_d = nc.dram_tensor("tri_d", [DH, 2 * DH], EDT, kind="ExternalInput")
    ones_c_d = nc.dram_tensor("ones_c_d", [DH, 32], EDT, kind="ExternalInput")
    ones_r_d = nc.dram_tensor("ones_r_d", [1, DH], F32R, kind="ExternalInput")
    ones_ra_d = nc.dram_tensor("ones_ra_d", [1, DH], ADT, kind="ExternalInput")
    outT_d = nc.dram_tensor("outT_d", [EH, S], ADT, kind="ExternalOutput")

    # 3D views with the 128-partition dim innermost on rows
    qT3 = qT[:].rearrange("(f p) s -> p f s", p=DH)
    kT3 = kT[:].rearrange("(f p) s -> p f s", p=DH)
    wq3 = wqT[:].rearrange("(f p) e -> p f e", p=DH)
    wk3 = wkT[:].rearrange("(f p) e -> p f e", p=DH)
    wv3 = wvT[:].rearrange("(f p) e -> p f e", p=DH)

    with TileContext(nc) as tc, nc.allow_low_precision(reason="fp8/bf16 ok"):
        with tc.tile_pool(name="persist", bufs=1) as persist:
            tri = persist.tile([DH, 2 * DH], EDT, tag="tri")
            kbias = persist.tile([DH, NKT], F32, tag="kbias")
            # ones_c: viewed [DH, 2, 16]; [:, :, 0:1] is the DoubleRow pair
            # lhsT (pair-dim stride 16B); [:, 0:1] flat is the single lhsT
            ones_c = persist.tile([DH, 32], EDT, tag="ones_c")
            ones_r = persist.tile([1, DH], F32R, tag="ones_r")
            ones_ra = persist.tile([1, DH], ADT, tag="ones_ra")
            bq_sb = persist.tile([DH, HPC], F32, tag="bq")
            bk_sb = persist.tile([DH, HPC], F32, tag="bk")
            bv_sb = persist.tile([1, EH], ADT, tag="bv")

            def persist_dmas():
                nc.sync.dma_start(tri[:], tri_d[:])
                nc.sync.dma_start(kbias[:], kbias_d[:])
                nc.sync.dma_start(ones_c[:], ones_c_d[:])
                nc.sync.dma_start(ones_r[:], ones_r_d[:])
                nc.sync.dma_start(ones_ra[:], ones_ra_d[:])
                nc.sync.dma_start(bq_sb[:], bq_d[:])
                nc.sync.dma_start(bk_sb[:], bk_d[:])
                nc.sync.dma_start(bv_sb[:], bv_d[:])

            ktS = persist.tile([DH, HPC * S], ADT, tag="ktS", name="ktS")
            vS = persist.tile([DH, NKT * EH], EDT, tag="vS", name="vS")
            qtS = persist.tile([DH, HPC * S], ADT, tag="qtS", name="qtS")
            wk_t = persist.tile([DH, NF * EH], IDT, tag="wk", name="wk")
            wv_t = persist.tile([DH, NF * EH], IDT, tag="wv", name="wv")
            wq_t = persist.tile([DH, NF * EH], IDT, tag="wq", name="wq")

            for _rep in range(reps):
                _rep_body(
                    nc, tc, scale, fast, has_bias, use_fp8,
                    persist_dmas if _rep == 0 else None,
                    kT3, qT3, wk3, wv3, wq3, outT_d,
                    ktS, vS, qtS, wk_t, wv_t, wq_t, _rep == 0,
                    tri, kbias, ones_c, ones_r, ones_ra,
                    bq_sb, bk_sb, bv_sb,
                    F32, F32R, ADT, IDT, EDT, EXP, IDENT,
                )

    _split_excess_waits(nc, max_waits=1, compute_max_waits=COMPUTE_MAX_WAITS)
    _CACHE[key] = nc
    return nc


def _dma_fgroups(nc, sb_tile, src3, interleave=None):
    """DMA a [p, NF, ncols] DRAM view into an SBUF tile [DH, NF*ncols],
    split into FG-sized f-groups so consumers can start early. If
    `interleave` is a second (tile, src3) pair, alternate its f-groups
    with the first's on the DMA queue."""
    view = sb_tile[:].rearrange("p (f c) -> p f c", f=NF)
    iview = None
    if interleave is not None:
        itile, isrc3 = interleave
        iview = itile[:].rearrange("p (f c) -> p f c", f=NF)
    first = True
    for g in range(0, NF, FG):
        if first and iview is not None:
            # split the very first groups in half so the first consumer
            # matmul starts ~1.5us earlier
            h = FG // 2
            nc.sync.dma_start(view[:, g : g + h, :], src3[:, g : g + h, :])
            nc.sync.dma_start(iview[:, g : g + h, :], isrc3[:, g : g + h, :])
            nc.sync.dma_start(view[:, g + h : g + FG, :], src3[:, g + h : g + FG, :])
            nc.sync.dma_start(iview[:, g + h : g + FG, :], isrc3[:, g + h : g + FG, :])
            first = False
            continue
        nc.sync.dma_start(view[:, g : g + FG, :], src3[:, g : g + FG, :])
        if iview is not None:
            nc.sync.dma_start(iview[:, g : g + FG, :], isrc3[:, g : g + FG, :])


def _proj_mms(nc, use_fp8, out_ps, w_view, x_view, wcols, xcols, extra_mm=None):
    """Accumulate out_ps += sum_f w[f][:, wcols].T @ x[f][:, xcols] over all
    NF f-tiles. fp8: DoubleRow over f-pairs; else per-f bf16 matmuls.
    w_view/x_view are [p, f, c] APs. extra_mm appends a final accumulating
    matmul (bias) before stop."""
    import concourse.mybir as mybir

    last_stop = extra_mm is None
    if use_fp8:
        for g in range(NF // 2):
            nc.tensor.matmul(
                out_ps,
                w_view[:, 2 * g : 2 * g + 2, wcols],
                x_view[:, 2 * g : 2 * g + 2, xcols],
                start=(g == 0),
                stop=(last_stop and g == NF // 2 - 1),
                perf_mode=mybir.MatmulPerfMode.DoubleRow,
            )
    else:
        for f in range(NF):
            nc.tensor.matmul(
                out_ps,
                w_view[:, f, wcols],
                x_view[:, f, xcols],
                start=(f == 0),
                stop=(last_stop and f == NF - 1),
            )
    if extra_mm is not None:
        extra_mm()


def _rep_body(
    nc, tc, scale, fast, has_bias, use_fp8, persist_dmas,
    kT3, qT3, wk3, wv3, wq3, outT_d,
    ktS, vS, qtS, wk_t, wv_t, wq_t, first_rep,
    tri, kbias, ones_c, ones_r, ones_ra,
    bq_sb, bk_sb, bv_sb,
    F32, F32R, ADT, IDT, EDT, EXP, IDENT,
):
    vS3 = vS[:].rearrange("p (kt e) -> p kt e", kt=NKT)
    ktS3 = ktS[:].rearrange("p (et s) -> p et s", et=HPC)
    qtS3 = qtS[:].rearrange("p (et s) -> p et s", et=HPC)
    # ---------------- Phase KV ----------------
    # one chunk pool spans KV and Q so qch DMAs overlap late-KV compute
    chp_cm = tc.tile_pool(name="ch", bufs=2)
    chp = chp_cm.__enter__()
    with tc.tile_pool(name="pkv", bufs=2, space="PSUM") as pkvp:
        wk_v = wk_t[:].rearrange("p (f e) -> p f e", f=NF)
        wv_v = wv_t[:].rearrange("p (f e) -> p f e", f=NF)
        kc0 = chp.tile([DH, NF * PC], IDT, tag="ch", name="kc0")
        if first_rep:
            # interleave wk with the first keys chunk so the first
            # projection matmul starts early instead of behind all the
            # weight DMA; wq loads up front too (persistent) so the Q
            # phase has no weight-space WAR wait at all
            _dma_fgroups(nc, wk_t, wk3, interleave=(kc0, kT3[:, :, 0:PC]))
            if persist_dmas is not None:
                persist_dmas()
            _dma_fgroups(nc, wv_t, wv3)
        else:
            _dma_fgroups(nc, kc0, kT3[:, :, 0:PC])
        for sc in range(NPC * scale.get("k", 1)):
            s0 = (sc % NPC) * PC
            if sc == 0:
                kc = kc0
            else:
                kc = chp.tile([DH, NF * PC], IDT, tag="ch", name="kc")
                if sc == 1 and first_rep:
                    # wq (persistent) rides along with the kc1 chunk so it
                    # neither delays kc1 nor leaves a WAR gap before Q
                    _dma_fgroups(
                        nc, kc, kT3[:, :, s0 : s0 + PC],
                        interleave=(wq_t, wq3),
                    )
                else:
                    _dma_fgroups(nc, kc, kT3[:, :, s0 : s0 + PC])
            kc_v = kc[:].rearrange("p (f s) -> p f s", f=NF)
            for et in range(0, HPC, 2):
                pk = pkvp.tile([DH, 2 * PC], F32, name="pk", tag="pk2")
                for half in range(2):
                    _proj_mms(
                        nc, use_fp8, pk[:, half * PC : (half + 1) * PC],
                        wk_v, kc_v,
                        slice((et + half) * DH, (et + half + 1) * DH),
                        slice(0, PC),
                    )
                if has_bias:
                    for half in range(2):
                        nc.scalar.activation(
                            ktS3[:, et + half, s0 : s0 + PC],
                            pk[:, half * PC : (half + 1) * PC], IDENT,
                            bias=bk_sb[:, et + half : et + half + 1],
                        )
                else:
                    # one strided copy lands both heads' chunks
                    nc.vector.tensor_copy(
                        ktS3[:, et : et + 2, s0 : s0 + PC],
                        pk[:].rearrange("p (two s) -> p two s", two=2),
                    )
            for sti in range(PC // DH):
                kt_idx = (s0 // DH) + sti
                pv = pkvp.tile([DH, 2 * PC], F32, name="pv", tag="pk2")
                for ec in range(EH // PC):
                    extra = None
                    if has_bias:
                        def extra(pv=pv, ec=ec):
                            nc.tensor.matmul(
                                pv[:, ec * PC : (ec + 1) * PC],
                                ones_ra[:],
                                bv_sb[:, ec * PC : (ec + 1) * PC],
                                start=False,
                                stop=True,
                            )
                    _proj_mms(
                        nc, use_fp8, pv[:, ec * PC : (ec + 1) * PC],
                        kc_v, wv_v,
                        slice(sti * DH, (sti + 1) * DH),
                        slice(ec * PC, (ec + 1) * PC),
                        extra_mm=extra,
                    )
                nc.vector.tensor_copy(vS3[:, kt_idx, :], pv[:])

    # ---------------- Phase Q ----------------
    with tc.tile_pool(name="pq", bufs=2, space="PSUM") as pqp:
        wq_v = wq_t[:].rearrange("p (f e) -> p f e", f=NF)
        qch0 = chp.tile([DH, NF * PC], IDT, tag="ch", name="qch0")
        _dma_fgroups(nc, qch0, qT3[:, :, 0:PC])
        for sc in range(NPC * scale.get("q", 1)):
            s0 = (sc % NPC) * PC
            if sc == 0:
                qch = qch0
            else:
                qch = chp.tile([DH, NF * PC], IDT, tag="ch", name="qch")
                _dma_fgroups(nc, qch, qT3[:, :, s0 : s0 + PC])
            qch_v = qch[:].rearrange("p (f s) -> p f s", f=NF)
            for et in range(0, HPC, 2):
                pq = pqp.tile([DH, 2 * PC], F32, name="pq", tag="pq2")
                for half in range(2):
                    _proj_mms(
                        nc, use_fp8, pq[:, half * PC : (half + 1) * PC],
                        wq_v, qch_v,
                        slice((et + half) * DH, (et + half + 1) * DH),
                        slice(0, PC),
                    )
                if has_bias:
                    for half in range(2):
                        nc.scalar.activation(
                            qtS3[:, et + half, s0 : s0 + PC],
                            pq[:, half * PC : (half + 1) * PC], IDENT,
                            bias=bq_sb[:, et + half : et + half + 1],
                        )
                else:
                    nc.vector.tensor_copy(
                        qtS3[:, et : et + 2, s0 : s0 + PC],
                        pq[:].rearrange("p (two s) -> p two s", two=2),
                    )

    chp_cm.__exit__(None, None, None)

    # ---------------- Phase attention ----------------
    _attention(
        nc, tc, fast, use_fp8, ktS3, vS3, qtS3, outT_d,
        tri, kbias, ones_c, ones_r, F32, F32R, ADT, EDT, EXP,
        scale.get("attn", 1),
    )


def _attention(
    nc, tc, fast, use_fp8, ktS3, vS3, qtS3, outT_d,
    tri, kbias, ones_c, ones_r, F32, F32R, ADT, EDT, EXP, attn_scale=1,
):
    import concourse.mybir as mybir

    DR = mybir.MatmulPerfMode.DoubleRow
    ones2 = ones_c[:].rearrange("p (two c) -> p two c", two=2)

    with tc.tile_pool(name="ex", bufs=4) as exp_p, \
         tc.tile_pool(name="tail", bufs=3) as tailp, \
         tc.tile_pool(name="outs", bufs=3) as outp_sb, \
         tc.tile_pool(name="ps_s", bufs=2, space="PSUM") as pss, \
         tc.tile_pool(name="ps_o", bufs=2, space="PSUM") as pso, \
         tc.tile_pool(name="ps_t", bufs=1, space="PSUM") as pst, \
         tc.tile_pool(name="ps_b", bufs=1, space="PSUM") as psb:
        # Software pipeline: for each pair-group of k-tiles, emit the score
        # matmuls + Exp, then the *previous* group's V/sum matmuls. The PE
        # queue is in-order, so this keeps PE busy on group g's scores and
        # group g-1's V-matmuls while ACT runs Exp(g).
        state = {}

        def emit_scores(h, qc, g):
            q0 = qc * QC
            nfull = 4 * qc
            kt0, kt1 = 2 * g, 2 * g + 1
            off0 = (kt0 - nfull) * DH if kt0 >= nfull else 0
            off1 = (kt1 - nfull) * DH if kt1 >= nfull else 0
            ps = pss.tile([DH, 2 * QC], F32, name="ps")
            nc.tensor.matmul(
                ps[:, off0:QC],
                ktS3[:, h, kt0 * DH : (kt0 + 1) * DH],
                qtS3[:, h, q0 + off0 : q0 + QC],
                start=True,
                stop=True,
            )
            nc.tensor.matmul(
                ps[:, QC + off1 : 2 * QC],
                ktS3[:, h, kt1 * DH : (kt1 + 1) * DH],
                qtS3[:, h, q0 + off1 : q0 + QC],
                start=True,
                stop=True,
            )
            ex = exp_p.tile([DH, 3 * QC], EDT, tag="ex", name="ex")
            if fast:
                # one Exp over both banks; cols outside the valid windows
                # hold exp(stale-PSUM) and are never read
                nc.scalar.activation(ex[:, 0 : 2 * QC], ps[:], EXP,
                                     scale=float(SCALE))
            else:
                nc.scalar.activation(
                    ex[:, off0:QC], ps[:, off0:QC], EXP,
                    bias=kbias[:, kt0 : kt0 + 1], scale=float(SCALE),
                )
                nc.scalar.activation(
                    ex[:, QC + off1 : 2 * QC],
                    ps[:, QC + off1 : 2 * QC], EXP,
                    bias=kbias[:, kt1 : kt1 + 1], scale=float(SCALE),
                )
            # causal triangle on the diagonal 128x128 blocks; for a band
            # pair the two blocks sit QC+DH apart -> one strided mul
            if kt0 >= nfull and kt1 >= nfull:
                both = ex[:, off0 : off0 + 2 * (QC + DH)].rearrange(
                    "p (two c) -> p two c", two=2
                )[:, :, 0:DH]
                tri2 = tri[:].rearrange("p (two c) -> p two c", two=2)
                nc.vector.tensor_mul(both, both, tri2)
            elif kt1 >= nfull:
                nc.vector.tensor_mul(
                    ex[:, QC + off1 : QC + off1 + DH],
                    ex[:, QC + off1 : QC + off1 + DH],
                    tri[:, 0:DH],
                )
            return (ex, off0, off1)

        def emit_pv(h, qc, g, scores):
            nkt = 4 * qc + 4
            nfull = 4 * qc
            ex, off0, off1 = scores
            po, psum = state[(h, qc)]
            kt0, kt1 = 2 * g, 2 * g + 1
            if use_fp8 and kt1 < nfull:
                # full-tile pair: one DoubleRow matmul contracts both k-tiles
                ex2 = ex[:, 0 : 2 * QC].rearrange("p (two q) -> p two q", two=2)
                nc.tensor.matmul(
                    po[:],
                    vS3[:, kt0 : kt0 + 2, h * DH : (h + 1) * DH],
                    ex2,
                    start=(kt0 == 0),
                    stop=(kt1 == nkt - 1),
                    perf_mode=DR,
                )
                nc.tensor.matmul(
                    psum[:],
                    ones2[:, :, 0:1],
                    ex2,
                    start=(kt0 == 0),
                    stop=(kt1 == nkt - 1),
                    perf_mode=DR,
                )
                return
            for kt, exs, off in (
                (kt0, ex[:, off0:QC], off0),
                (kt1, ex[:, QC + off1 : 2 * QC], off1),
            ):
                nc.tensor.matmul(
                    po[:, off:QC],
                    vS3[:, kt, h * DH : (h + 1) * DH],
                    exs,
                    start=(kt == 0),
                    stop=(kt == nkt - 1),
                )
                nc.tensor.matmul(
                    psum[:, off:QC],
                    ones_c[:, 0:1],
                    exs,
                    start=(kt == 0),
                    stop=(kt == nkt - 1),
                )

        def emit_tail(h, qc, oth):
            q0 = qc * QC
            po, psum = state.pop((h, qc))
            rec = tailp.tile([1, QC], F32R, tag="rec", name="rec")
            nc.vector.reciprocal(rec[:], psum[:])
            pbc = psb.tile([DH, QC], F32, name="pbc")
            nc.tensor.matmul(pbc[:], ones_r[:], rec[:], start=True, stop=True)
            bcs = tailp.tile([DH, QC], F32, tag="bcs", name="bcs")
            nc.vector.tensor_copy(bcs[:], pbc[:])
            nc.vector.tensor_mul(oth[:, q0 : q0 + QC], po[:], bcs[:])

        steps = []
        for hh in range(HPC * attn_scale):
            h = hh % HPC
            for qc in range(NQC):
                for g in range(2 * qc + 2):
                    steps.append((hh, h, qc, g))

        oth_t = {}
        pending = None
        for hh, h, qc, g in steps:
            if g == 0:
                if qc == 0:
                    oth_t[hh] = outp_sb.tile([DH, S], ADT, tag="oth", name="oth")
                state[(h, qc)] = (
                    pso.tile([DH, QC], F32, name="po"),
                    pst.tile([1, QC], F32, name="psum"),
                )
            scores = emit_scores(h, qc, g)
            if pending is not None:
                phh, ph, pqc, pg, pscores = pending
                emit_pv(ph, pqc, pg, pscores)
                if pg == 2 * pqc + 1:
                    emit_tail(ph, pqc, oth_t[phh])
                    if pqc == NQC - 1:
                        nc.sync.dma_start(
                            outT_d[ph * DH : (ph + 1) * DH, :], oth_t.pop(phh)[:]
                        )
            pending = (hh, h, qc, g, scores)
        if pending is not None:
            phh, ph, pqc, pg, pscores = pending
            emit_pv(ph, pqc, pg, pscores)
            emit_tail(ph, pqc, oth_t[phh])
            nc.sync.dma_start(
                outT_d[ph * DH : (ph + 1) * DH, :], oth_t.pop(phh)[:]
            )


def _host_prep(queries, keys, Wq, bq, Wk, bk, Wv, bv, use_fp8=None):
    """Build the 8 per-core input maps (host-side shard + layout prep)."""
    import ml_dtypes

    if use_fp8 is None:
        use_fp8 = USE_FP8
    adt = ml_dtypes.bfloat16
    idt = ml_dtypes.float8_e4m3 if use_fp8 else adt
    edt = ml_dtypes.float8_e4m3 if use_fp8 else adt
    queries = np.ascontiguousarray(queries, dtype=np.float32)
    keys = np.ascontiguousarray(keys, dtype=np.float32)

    qTa = np.ascontiguousarray(queries.transpose(0, 2, 1)).astype(idt)
    kTa = np.ascontiguousarray(keys.transpose(0, 2, 1)).astype(idt)
    WqT = np.ascontiguousarray(np.asarray(Wq, np.float32).T).astype(idt)  # [f, e]
    WkT = np.ascontiguousarray(np.asarray(Wk, np.float32).T).astype(idt)
    WvT = np.ascontiguousarray(np.asarray(Wv, np.float32).T).astype(idt)
    bq = np.asarray(bq, np.float32)
    bk = np.asarray(bk, np.float32)
    bv = np.asarray(bv, np.float32)

    # key padding mask -> additive bias per (b, k): 0 keep, -1e30 mask
    ksum = keys.sum(axis=-1)  # [B, S]
    kbias_all = np.where(ksum != 0.0, np.float32(0), NEG_BIAS).astype(np.float32)

    # causal triangle for diagonal 128x128 blocks: keep iff q_local >= k_local
    tri1 = (np.arange(DH)[None, :] >= np.arange(DH)[:, None]).astype(edt)
    tri = np.concatenate([tri1, tri1], axis=1)

    ones_c = np.ones((DH, 32), edt)
    ones_r = np.ones((1, DH), np.float32)

    in_maps = []
    for c in range(NCORES):
        b, hg = divmod(c, 2)
        e0 = hg * EH
        in_maps.append(
            {
                "qT": qTa[b],
                "kT": kTa[b],
                "wqT": np.ascontiguousarray(WqT[:, e0 : e0 + EH]),
                "wkT": np.ascontiguousarray(WkT[:, e0 : e0 + EH]),
                "wvT": np.ascontiguousarray(WvT[:, e0 : e0 + EH]),
                "bq_d": np.ascontiguousarray(bq[e0 : e0 + EH].reshape(HPC, DH).T),
                "bk_d": np.ascontiguousarray(bk[e0 : e0 + EH].reshape(HPC, DH).T),
                "bv_d": np.ascontiguousarray(
                    bv[e0 : e0 + EH].reshape(1, EH)
                ).astype(adt),
                "kbias_d": np.ascontiguousarray(kbias_all[b].reshape(NKT, DH).T),
                "tri_d": tri,
                "ones_c_d": ones_c,
                "ones_r_d": ones_r,
                "ones_ra_d": ones_r.astype(adt),
            }
        )
    return in_maps


def _assemble(results, queries):
    """results: 8 dicts with outT_d [EH, S] bf16 -> full [B, S, HID] f32
    with the queries residual added on host."""
    out = np.empty((B, S, HID), np.float32)
    for c in range(NCORES):
        b, hg = divmod(c, 2)
        out[b, :, hg * EH : (hg + 1) * EH] = (
            results[c]["outT_d"].astype(np.float32).T
            + queries[b, :, hg * EH : (hg + 1) * EH]
        )
    return out


def kernel(**inputs):
    from concourse.bass_utils import run_bass_kernel_spmd

    queries = np.asarray(inputs["queries"], np.float32)
    keys = np.asarray(inputs["keys"], np.float32)
    # fast path valid unless some key row is exactly zero-sum (padding)
    fast = not bool(np.any(keys.sum(axis=-1) == 0.0))
    has_bias = bool(
        np.any(np.asarray(inputs["bq"]))
        or np.any(np.asarray(inputs["bk"]))
        or np.any(np.asarray(inputs["bv"]))
    )
    nc = _build(fast=fast, has_bias=has_bias)
    in_maps = _host_prep(**inputs)
    res = run_bass_kernel_spmd(nc, in_maps, core_ids=list(range(NCORES)))
    kernel.last_results = res
    return _assemble(res.results, queries)


# revision 7
# speedup vs baseline: 1.3009x; 1.3009x over previous
"""Trainium2 Bass kernel for nn_MultiHeadAttention_88210038326473 (v3, fp8).

Reference computation (B=4, S=2048, HID=2048, H=16, DH=128):
    Q = queries @ Wq.T + bq ; K = keys @ Wk.T + bk ; V = keys @ Wv.T + bv
    per-head scores = Qh Kh^T / sqrt(HID), key-padding + causal mask,
    softmax, out = attn @ Vh, concat heads, + queries residual.

Sharding: 8 cores = 4 batches x 2 head-groups (8 heads each). Each core
computes attn_out[b, :, hg*1024:(hg+1)*1024] ([1024, 2048] bf16,
transposed); host transposes back, adds the queries residual in fp32,
and assembles.

v3 vs v2: fp8(e4m3) everywhere the error budget allows, validated by
host-side emulation at rel_err ~4e-3 (budget 2e-2; the fp32 queries
residual dominates the output norm, diluting attention error ~12x):
  - projection inputs + weights in fp8, contracted 2 f-tiles per
    DoubleRow matmul (2x PE throughput),
  - probabilities (Exp output) and V in fp8; the attn V-matmul and the
    row-sum matmul consume full k-tile PAIRS per DoubleRow matmul,
  - scores stay bf16 (contraction=128 cannot DoubleRow; fp8 wouldn't be
    faster and costs accuracy).
K^T/V/Q^T all SBUF-resident, paired Exp over 2-bank PSUM groups,
software-pipelined attention (scores one pair-group ahead of V/sum),
residual add on host.
"""

import math

import numpy as np

B, S, HID, H, DH = 4, 2048, 2048, 16, 128
NCORES = 8
HPC = 8          # heads per core
EH = HPC * DH    # 1024 e-dims per core
SCALE = 1.0 / math.sqrt(HID)
QC = 512         # attention q-chunk
NQC = S // QC    # 4
NKT = S // DH    # 16 k-tiles
NF = HID // DH   # 16 f-tiles (contraction)
PC = 512         # projection s-chunk (matmul moving N)
NPC = S // PC    # 4
FG = 4           # f-tiles per DMA group
NEG_BIAS = np.float32(-1.0e30)
USE_FP8 = True
COMPUTE_MAX_WAITS = 1


CTRL_OPS = ("InstDrain", "InstNoOp", "InstEventSemaphore", "InstISA")


def _split_excess_waits(nc, max_waits=1, compute_max_waits=None):
    """walrus in this container rejects >1 sem-wait per CTRL-class
    instruction. Move excess waits onto preceding NoOps on the same
    engine."""
    import concourse.mybir as mybir

    if compute_max_waits is None:
        compute_max_waits = max_waits
    n_split = 0
    for fn in nc.m.functions:
        for blk in fn.blocks:
            insts = list(blk.instructions)
            out = []
            changed = False
            for ins in insts:
                lim = (
                    max_waits
                    if type(ins).__name__ in CTRL_OPS
                    else compute_max_waits
                )
                si = ins.sync_info
                if si is not None and si.on_wait and len(si.on_wait) > lim:
                    waits = list(si.on_wait)
                    carriers, rest = waits[:-lim], waits[-lim:]
                    for i in range(0, len(carriers), max_waits):
                        chunk = carriers[i : i + max_waits]
                        out.append(
                            mybir.InstNoOp(
                                name=f"{ins.name}-ws{i}",
                                engine=ins.engine,
                                bass_nofuse=True,
                                sync_info=mybir.SyncInfo(on_wait=chunk, on_update=[]),
                            )
                        )
                        n_split += 1
                    ins.sync_info = mybir.SyncInfo(
                        on_wait=rest, on_update=list(si.on_update)
                    )
                    changed = True
                out.append(ins)
            if changed:
                blk.instructions = out
    return n_split


_CACHE = {}


def _build(fast=True, has_bias=False, reps=1, use_fp8=None, scale=None):
    """Build the (core-uniform) Bass program. Returns nc.

    fast=True drops the key-padding bias from the exp (valid when no key
    is padding -- checked on host). has_bias=False skips projection
    biases (valid when bq=bk=bv=0 -- checked on host). use_fp8 switches
    projection inputs/weights + probabilities/V to fp8e4 with DoubleRow
    matmuls. reps>1 repeats the whole body; scale={"k"/"q"/"attn": n}
    repeats a phase (timing instruments)."""
    scale = scale or {}
    if use_fp8 is None:
        use_fp8 = USE_FP8
    key = ("nc", fast, has_bias, reps, use_fp8, tuple(sorted(scale.items())))
    if key in _CACHE:
        return _CACHE[key]

    import concourse.bass as bass
    import concourse.mybir as mybir
    from concourse.tile import TileContext

    F32 = mybir.dt.float32
    F32R = mybir.dt.float32r
    ADT = mybir.dt.bfloat16
    IDT = mybir.dt.float8e4 if use_fp8 else ADT  # proj inputs + weights
    EDT = mybir.dt.float8e4 if use_fp8 else ADT  # probabilities + V
    EXP = mybir.ActivationFunctionType.Exp
    IDENT = mybir.ActivationFunctionType.Identity

    nc = bass.Bass("TRN2", target_bir_lowering=False, debug=False)

    qT = nc.dram_tensor("qT", [HID, S], IDT, kind="ExternalInput")
    kT = nc.dram_tensor("kT", [HID, S], IDT, kind="ExternalInput")
    wqT = nc.dram_tensor("wqT", [HID, EH], IDT, kind="ExternalInput")
    wkT = nc.dram_tensor("wkT", [HID, EH], IDT, kind="ExternalInput")
    wvT = nc.dram_tensor("wvT", [HID, EH], IDT, kind="ExternalInput")
    bq_d = nc.dram_tensor("bq_d", [DH, HPC], F32, kind="ExternalInput")
    bk_d = nc.dram_tensor("bk_d", [DH, HPC], F32, kind="ExternalInput")
    bv_d = nc.dram_tensor("bv_d", [1, EH], ADT, kind="ExternalInput")
    kbias_d = nc.dram_tensor("kbias_d", [DH, NKT], F32, kind="ExternalInput")
    tri_d = nc.dram_tensor("tri_d", [DH, 2 * DH], EDT, kind="ExternalInput")
    ones_c_d = nc.dram_tensor("ones_c_d", [DH, 32], EDT, kind="ExternalInput")
    ones_r_d = nc.dram_tensor("ones_r_d", [1, DH], F32R, kind="ExternalInput")
    ones_ra_d = nc.dram_tensor("ones_ra_d", [1, DH], ADT, kind="ExternalInput")
    outT_d = nc.dram_tensor("outT_d", [EH, S], ADT, kind="ExternalOutput")

    # 3D views with the 128-partition dim innermost on rows
    qT3 = qT[:].rearrange("(f p) s -> p f s", p=DH)
    kT3 = kT[:].rearrange("(f p) s -> p f s", p=DH)
    wq3 = wqT[:].rearrange("(f p) e -> p f e", p=DH)
    wk3 = wkT[:].rearrange("(f p) e -> p f e", p=DH)
    wv3 = wvT[:].rearrange("(f p) e -> p f e", p=DH)

    with TileContext(nc) as tc, nc.allow_low_precision(reason="fp8/bf16 ok"):
        with tc.tile_pool(name="persist", bufs=1) as persist:
            tri = persist.tile([DH, 2 * DH], EDT, tag="tri")
            kbias = persist.tile([DH, NKT], F32, tag="kbias")
            # ones_c: viewed [DH, 2, 16]; [:, :, 0:1] is the DoubleRow pair
            # lhsT (pair-dim stride 16B); [:, 0:1] flat is the single lhsT
            ones_c = persist.tile([DH, 32], EDT, tag="ones_c")
            ones_r = persist.tile([1, DH], F32R, tag="ones_r")
            ones_ra = persist.tile([1, DH], ADT, tag="ones_ra")
            bq_sb = persist.tile([DH, HPC], F32, tag="bq")
            bk_sb = persist.tile([DH, HPC], F32, tag="bk")
            bv_sb = persist.tile([1, EH], ADT, tag="bv")

            def persist_dmas():
                # only load what this build variant actually reads
                nc.sync.dma_start(tri[:], tri_d[:])
                nc.sync.dma_start(ones_c[:], ones_c_d[:])
                nc.sync.dma_start(ones_r[:], ones_r_d[:])
                if not fast:
                    nc.sync.dma_start(kbias[:], kbias_d[:])
                if has_bias:
                    nc.sync.dma_start(ones_ra[:], ones_ra_d[:])
                    nc.sync.dma_start(bq_sb[:], bq_d[:])
                    nc.sync.dma_start(bk_sb[:], bk_d[:])
                    nc.sync.dma_start(bv_sb[:], bv_d[:])

            ktS = persist.tile([DH, HPC * S], ADT, tag="ktS", name="ktS")
            vS = persist.tile([DH, NKT * EH], EDT, tag="vS", name="vS")
            qtS = persist.tile([DH, HPC * S], ADT, tag="qtS", name="qtS")
            wk_t = persist.tile([DH, NF * EH], IDT, tag="wk", name="wk")
            wv_t = persist.tile([DH, NF * EH], IDT, tag="wv", name="wv")
            wq_t = persist.tile([DH, NF * EH], IDT, tag="wq", name="wq")

            for _rep in range(reps):
                _rep_body(
                    nc, tc, scale, fast, has_bias, use_fp8,
                    persist_dmas if _rep == 0 else None,
                    kT3, qT3, wk3, wv3, wq3, outT_d,
                    ktS, vS, qtS, wk_t, wv_t, wq_t, _rep == 0,
                    tri, kbias, ones_c, ones_r, ones_ra,
                    bq_sb, bk_sb, bv_sb,
                    F32, F32R, ADT, IDT, EDT, EXP, IDENT,
                )

    _split_excess_waits(nc, max_waits=1, compute_max_waits=COMPUTE_MAX_WAITS)
    _CACHE[key] = nc
    return nc


def _dma_fgroups(nc, sb_tile, src3, interleave=None):
    """DMA a [p, NF, ncols] DRAM view into an SBUF tile [DH, NF*ncols],
    split into FG-sized f-groups so consumers can start early. If
    `interleave` is a second (tile, src3) pair, alternate its f-groups
    with the first's on the DMA queue."""
    view = sb_tile[:].rearrange("p (f c) -> p f c", f=NF)
    iview = None
    if interleave is not None:
        itile, isrc3 = interleave
        iview = itile[:].rearrange("p (f c) -> p f c", f=NF)
    first = True
    for g in range(0, NF, FG):
        if first and iview is not None:
            # split the very first groups in half so the first consumer
            # matmul starts ~1.5us earlier
            h = FG // 2
            nc.sync.dma_start(view[:, g : g + h, :], src3[:, g : g + h, :])
            nc.sync.dma_start(iview[:, g : g + h, :], isrc3[:, g : g + h, :])
            nc.sync.dma_start(view[:, g + h : g + FG, :], src3[:, g + h : g + FG, :])
            nc.sync.dma_start(iview[:, g + h : g + FG, :], isrc3[:, g + h : g + FG, :])
            first = False
            continue
        nc.sync.dma_start(view[:, g : g + FG, :], src3[:, g : g + FG, :])
        if iview is not None:
            nc.sync.dma_start(iview[:, g : g + FG, :], isrc3[:, g : g + FG, :])


def _proj_mms(nc, use_fp8, out_ps, w_view, x_view, wcols, xcols, extra_mm=None):
    """Accumulate out_ps += sum_f w[f][:, wcols].T @ x[f][:, xcols] over all
    NF f-tiles. fp8: DoubleRow over f-pairs; else per-f bf16 matmuls.
    w_view/x_view are [p, f, c] APs. extra_mm appends a final accumulating
    matmul (bias) before stop."""
    import concourse.mybir as mybir

    last_stop = extra_mm is None
    if use_fp8:
        for g in range(NF // 2):
            nc.tensor.matmul(
                out_ps,
                w_view[:, 2 * g : 2 * g + 2, wcols],
                x_view[:, 2 * g : 2 * g + 2, xcols],
                start=(g == 0),
                stop=(last_stop and g == NF // 2 - 1),
                perf_mode=mybir.MatmulPerfMode.DoubleRow,
            )
    else:
        for f in range(NF):
            nc.tensor.matmul(
                out_ps,
                w_view[:, f, wcols],
                x_view[:, f, xcols],
                start=(f == 0),
                stop=(last_stop and f == NF - 1),
            )
    if extra_mm is not None:
        extra_mm()


def _rep_body(
    nc, tc, scale, fast, has_bias, use_fp8, persist_dmas,
    kT3, qT3, wk3, wv3, wq3, outT_d,
    ktS, vS, qtS, wk_t, wv_t, wq_t, first_rep,
    tri, kbias, ones_c, ones_r, ones_ra,
    bq_sb, bk_sb, bv_sb,
    F32, F32R, ADT, IDT, EDT, EXP, IDENT,
):
    vS3 = vS[:].rearrange("p (kt e) -> p kt e", kt=NKT)
    ktS3 = ktS[:].rearrange("p (et s) -> p et s", et=HPC)
    qtS3 = qtS[:].rearrange("p (et s) -> p et s", et=HPC)
    # ---------------- Phase KV ----------------
    # one chunk pool spans KV and Q so qch DMAs overlap late-KV compute
    chp_cm = tc.tile_pool(name="ch", bufs=2)
    chp = chp_cm.__enter__()
    with tc.tile_pool(name="pkv", bufs=2, space="PSUM") as pkvp:
        wk_v = wk_t[:].rearrange("p (f e) -> p f e", f=NF)
        wv_v = wv_t[:].rearrange("p (f e) -> p f e", f=NF)
        kc0 = chp.tile([DH, NF * PC], IDT, tag="ch", name="kc0")
        if first_rep:
            # interleave wk with the first keys chunk so the first
            # projection matmul starts early instead of behind all the
            # weight DMA; wq loads up front too (persistent) so the Q
            # phase has no weight-space WAR wait at all
            _dma_fgroups(nc, wk_t, wk3, interleave=(kc0, kT3[:, :, 0:PC]))
            if persist_dmas is not None:
                persist_dmas()
            _dma_fgroups(nc, wv_t, wv3)
        else:
            _dma_fgroups(nc, kc0, kT3[:, :, 0:PC])
        for sc in range(NPC * scale.get("k", 1)):
            s0 = (sc % NPC) * PC
            if sc == 0:
                kc = kc0
            else:
                kc = chp.tile([DH, NF * PC], IDT, tag="ch", name="kc")
                if sc == 1 and first_rep:
                    # wq (persistent) rides along with the kc1 chunk so it
                    # neither delays kc1 nor leaves a WAR gap before Q
                    _dma_fgroups(
                        nc, kc, kT3[:, :, s0 : s0 + PC],
                        interleave=(wq_t, wq3),
                    )
                else:
                    _dma_fgroups(nc, kc, kT3[:, :, s0 : s0 + PC])
            kc_v = kc[:].rearrange("p (f s) -> p f s", f=NF)
            for et in range(0, HPC, 2):
                pk = pkvp.tile([DH, 2 * PC], F32, name="pk", tag="pk2")
                for half in range(2):
                    _proj_mms(
                        nc, use_fp8, pk[:, half * PC : (half + 1) * PC],
                        wk_v, kc_v,
                        slice((et + half) * DH, (et + half + 1) * DH),
                        slice(0, PC),
                    )
                if has_bias:
                    for half in range(2):
                        nc.scalar.activation(
                            ktS3[:, et + half, s0 : s0 + PC],
                            pk[:, half * PC : (half + 1) * PC], IDENT,
                            bias=bk_sb[:, et + half : et + half + 1],
                        )
                else:
                    # one strided copy lands both heads' chunks
                    nc.vector.tensor_copy(
                        ktS3[:, et : et + 2, s0 : s0 + PC],
                        pk[:].rearrange("p (two s) -> p two s", two=2),
                    )
            for sti in range(PC // DH):
                kt_idx = (s0 // DH) + sti
                pv = pkvp.tile([DH, 2 * PC], F32, name="pv", tag="pk2")
                for ec in range(EH // PC):
                    extra = None
                    if has_bias:
                        def extra(pv=pv, ec=ec):
                            nc.tensor.matmul(
                                pv[:, ec * PC : (ec + 1) * PC],
                                ones_ra[:],
                                bv_sb[:, ec * PC : (ec + 1) * PC],
                                start=False,
                                stop=True,
                            )
                    _proj_mms(
                        nc, use_fp8, pv[:, ec * PC : (ec + 1) * PC],
                        kc_v, wv_v,
                        slice(sti * DH, (sti + 1) * DH),
                        slice(ec * PC, (ec + 1) * PC),
                        extra_mm=extra,
                    )
                nc.vector.tensor_copy(vS3[:, kt_idx, :], pv[:])

    # ---------------- Phase Q ----------------
    with tc.tile_pool(name="pq", bufs=2, space="PSUM") as pqp:
        wq_v = wq_t[:].rearrange("p (f e) -> p f e", f=NF)
        qch0 = chp.tile([DH, NF * PC], IDT, tag="ch", name="qch0")
        _dma_fgroups(nc, qch0, qT3[:, :, 0:PC])
        for sc in range(NPC * scale.get("q", 1)):
            s0 = (sc % NPC) * PC
            if sc == 0:
                qch = qch0
            else:
                qch = chp.tile([DH, NF * PC], IDT, tag="ch", name="qch")
                _dma_fgroups(nc, qch, qT3[:, :, s0 : s0 + PC])
            qch_v = qch[:].rearrange("p (f s) -> p f s", f=NF)
            for et in range(0, HPC, 2):
                pq = pqp.tile([DH, 2 * PC], F32, name="pq", tag="pq2")
                for half in range(2):
                    _proj_mms(
                        nc, use_fp8, pq[:, half * PC : (half + 1) * PC],
                        wq_v, qch_v,
                        slice((et + half) * DH, (et + half + 1) * DH),
                        slice(0, PC),
                    )
                if has_bias:
                    for half in range(2):
                        nc.scalar.activation(
                            qtS3[:, et + half, s0 : s0 + PC],
                            pq[:, half * PC : (half + 1) * PC], IDENT,
                            bias=bq_sb[:, et + half : et + half + 1],
                        )
                else:
                    nc.vector.tensor_copy(
                        qtS3[:, et : et + 2, s0 : s0 + PC],
                        pq[:].rearrange("p (two s) -> p two s", two=2),
                    )

    chp_cm.__exit__(None, None, None)

    # ---------------- Phase attention ----------------
    _attention(
        nc, tc, fast, use_fp8, ktS3, vS3, qtS3, outT_d,
        tri, kbias, ones_c, ones_r, F32, F32R, ADT, EDT, EXP,
        scale.get("attn", 1),
    )


def _attention(
    nc, tc, fast, use_fp8, ktS3, vS3, qtS3, outT_d,
    tri, kbias, ones_c, ones_r, F32, F32R, ADT, EDT, EXP, attn_scale=1,
):
    import concourse.mybir as mybir

    DR = mybir.MatmulPerfMode.DoubleRow
    ones2 = ones_c[:].rearrange("p (two c) -> p two c", two=2)

    with tc.tile_pool(name="ex", bufs=4) as exp_p, \
         tc.tile_pool(name="tail", bufs=3) as tailp, \
         tc.tile_pool(name="outs", bufs=3) as outp_sb, \
         tc.tile_pool(name="ps_s", bufs=2, space="PSUM") as pss, \
         tc.tile_pool(name="ps_o", bufs=2, space="PSUM") as pso, \
         tc.tile_pool(name="ps_t", bufs=1, space="PSUM") as pst, \
         tc.tile_pool(name="ps_b", bufs=1, space="PSUM") as psb:
        # Software pipeline: for each pair-group of k-tiles, emit the score
        # matmuls + Exp, then the *previous* group's V/sum matmuls. The PE
        # queue is in-order, so this keeps PE busy on group g's scores and
        # group g-1's V-matmuls while ACT runs Exp(g).
        state = {}

        def emit_scores(h, qc, g):
            q0 = qc * QC
            nfull = 4 * qc
            kt0, kt1 = 2 * g, 2 * g + 1
            off0 = (kt0 - nfull) * DH if kt0 >= nfull else 0
            off1 = (kt1 - nfull) * DH if kt1 >= nfull else 0
            ps = pss.tile([DH, 2 * QC], F32, name="ps")
            nc.tensor.matmul(
                ps[:, off0:QC],
                ktS3[:, h, kt0 * DH : (kt0 + 1) * DH],
                qtS3[:, h, q0 + off0 : q0 + QC],
                start=True,
                stop=True,
            )
            nc.tensor.matmul(
                ps[:, QC + off1 : 2 * QC],
                ktS3[:, h, kt1 * DH : (kt1 + 1) * DH],
                qtS3[:, h, q0 + off1 : q0 + QC],
                start=True,
                stop=True,
            )
            ex = exp_p.tile([DH, 3 * QC], EDT, tag="ex", name="ex")
            if fast:
                # one Exp over both banks; cols outside the valid windows
                # hold exp(stale-PSUM) and are never read
                nc.scalar.activation(ex[:, 0 : 2 * QC], ps[:], EXP,
                                     scale=float(SCALE))
            else:
                nc.scalar.activation(
                    ex[:, off0:QC], ps[:, off0:QC], EXP,
                    bias=kbias[:, kt0 : kt0 + 1], scale=float(SCALE),
                )
                nc.scalar.activation(
                    ex[:, QC + off1 : 2 * QC],
                    ps[:, QC + off1 : 2 * QC], EXP,
                    bias=kbias[:, kt1 : kt1 + 1], scale=float(SCALE),
                )
            # causal triangle on the diagonal 128x128 blocks; for a band
            # pair the two blocks sit QC+DH apart -> one strided mul
            if kt0 >= nfull and kt1 >= nfull:
                both = ex[:, off0 : off0 + 2 * (QC + DH)].rearrange(
                    "p (two c) -> p two c", two=2
                )[:, :, 0:DH]
                tri2 = tri[:].rearrange("p (two c) -> p two c", two=2)
                nc.vector.tensor_mul(both, both, tri2)
            elif kt1 >= nfull:
                nc.vector.tensor_mul(
                    ex[:, QC + off1 : QC + off1 + DH],
                    ex[:, QC + off1 : QC + off1 + DH],
                    tri[:, 0:DH],
                )
            return (ex, off0, off1)

        def emit_pv(h, qc, g, scores):
            nkt = 4 * qc + 4
            nfull = 4 * qc
            ex, off0, off1 = scores
            po, psum = state[(h, qc)]
            kt0, kt1 = 2 * g, 2 * g + 1
            if use_fp8 and kt1 < nfull:
                # full-tile pair: one DoubleRow matmul contracts both k-tiles
                ex2 = ex[:, 0 : 2 * QC].rearrange("p (two q) -> p two q", two=2)
                nc.tensor.matmul(
                    po[:],
                    vS3[:, kt0 : kt0 + 2, h * DH : (h + 1) * DH],
                    ex2,
                    start=(kt0 == 0),
                    stop=(kt1 == nkt - 1),
                    perf_mode=DR,
                )
                nc.tensor.matmul(
                    psum[:],
                    ones2[:, :, 0:1],
                    ex2,
                    start=(kt0 == 0),
                    stop=(kt1 == nkt - 1),
                    perf_mode=DR,
                )
                return
            for kt, exs, off in (
                (kt0, ex[:, off0:QC], off0),
                (kt1, ex[:, QC + off1 : 2 * QC], off1),
            ):
                nc.tensor.matmul(
                    po[:, off:QC],
                    vS3[:, kt, h * DH : (h + 1) * DH],
                    exs,
                    start=(kt == 0),
                    stop=(kt == nkt - 1),
                )
                nc.tensor.matmul(
                    psum[:, off:QC],
                    ones_c[:, 0:1],
                    exs,
                    start=(kt == 0),
                    stop=(kt == nkt - 1),
                )

        def emit_tail(h, qc, oth):
            q0 = qc * QC
            po, psum = state.pop((h, qc))
            rec = tailp.tile([1, QC], F32R, tag="rec", name="rec")
            nc.vector.reciprocal(rec[:], psum[:])
            pbc = psb.tile([DH, QC], F32, name="pbc")
            nc.tensor.matmul(pbc[:], ones_r[:], rec[:], start=True, stop=True)
            bcs = tailp.tile([DH, QC], F32, tag="bcs", name="bcs")
            nc.vector.tensor_copy(bcs[:], pbc[:])
            nc.vector.tensor_mul(oth[:, q0 : q0 + QC], po[:], bcs[:])

        steps = []
        for hh in range(HPC * attn_scale):
            h = hh % HPC
            for qc in range(NQC):
                for g in range(2 * qc + 2):
                    steps.append((hh, h, qc, g))

        oth_t = {}
        pending = None
        for hh, h, qc, g in steps:
            if g == 0:
                if qc == 0:
                    oth_t[hh] = outp_sb.tile([DH, S], ADT, tag="oth", name="oth")
                state[(h, qc)] = (
                    pso.tile([DH, QC], F32, name="po"),
                    pst.tile([1, QC], F32, name="psum"),
                )
            scores = emit_scores(h, qc, g)
            if pending is not None:
                phh, ph, pqc, pg, pscores = pending
                emit_pv(ph, pqc, pg, pscores)
                if pg == 2 * pqc + 1:
                    emit_tail(ph, pqc, oth_t[phh])
                    if pqc == NQC - 1:
                        nc.sync.dma_start(
                            outT_d[ph * DH : (ph + 1) * DH, :], oth_t.pop(phh)[:]
                        )
            pending = (hh, h, qc, g, scores)
        if pending is not None:
            phh, ph, pqc, pg, pscores = pending
            emit_pv(ph, pqc, pg, pscores)
            emit_tail(ph, pqc, oth_t[phh])
            nc.sync.dma_start(
                outT_d[ph * DH : (ph + 1) * DH, :], oth_t.pop(phh)[:]
            )


def _host_prep(queries, keys, Wq, bq, Wk, bk, Wv, bv, use_fp8=None):
    """Build the 8 per-core input maps (host-side shard + layout prep)."""
    import ml_dtypes

    if use_fp8 is None:
        use_fp8 = USE_FP8
    adt = ml_dtypes.bfloat16
    idt = ml_dtypes.float8_e4m3 if use_fp8 else adt
    edt = ml_dtypes.float8_e4m3 if use_fp8 else adt
    queries = np.ascontiguousarray(queries, dtype=np.float32)
    keys = np.ascontiguousarray(keys, dtype=np.float32)

    qTa = np.ascontiguousarray(queries.transpose(0, 2, 1)).astype(idt)
    kTa = np.ascontiguousarray(keys.transpose(0, 2, 1)).astype(idt)
    WqT = np.ascontiguousarray(np.asarray(Wq, np.float32).T).astype(idt)  # [f, e]
    WkT = np.ascontiguousarray(np.asarray(Wk, np.float32).T).astype(idt)
    WvT = np.ascontiguousarray(np.asarray(Wv, np.float32).T).astype(idt)
    bq = np.asarray(bq, np.float32)
    bk = np.asarray(bk, np.float32)
    bv = np.asarray(bv, np.float32)

    # key padding mask -> additive bias per (b, k): 0 keep, -1e30 mask
    ksum = keys.sum(axis=-1)  # [B, S]
    kbias_all = np.where(ksum != 0.0, np.float32(0), NEG_BIAS).astype(np.float32)

    # causal triangle for diagonal 128x128 blocks: keep iff q_local >= k_local
    tri1 = (np.arange(DH)[None, :] >= np.arange(DH)[:, None]).astype(edt)
    tri = np.concatenate([tri1, tri1], axis=1)

    ones_c = np.ones((DH, 32), edt)
    ones_r = np.ones((1, DH), np.float32)

    in_maps = []
    for c in range(NCORES):
        b, hg = divmod(c, 2)
        e0 = hg * EH
        in_maps.append(
            {
                "qT": qTa[b],
                "kT": kTa[b],
                "wqT": np.ascontiguousarray(WqT[:, e0 : e0 + EH]),
                "wkT": np.ascontiguousarray(WkT[:, e0 : e0 + EH]),
                "wvT": np.ascontiguousarray(WvT[:, e0 : e0 + EH]),
                "bq_d": np.ascontiguousarray(bq[e0 : e0 + EH].reshape(HPC, DH).T),
                "bk_d": np.ascontiguousarray(bk[e0 : e0 + EH].reshape(HPC, DH).T),
                "bv_d": np.ascontiguousarray(
                    bv[e0 : e0 + EH].reshape(1, EH)
                ).astype(adt),
                "kbias_d": np.ascontiguousarray(kbias_all[b].reshape(NKT, DH).T),
                "tri_d": tri,
                "ones_c_d": ones_c,
                "ones_r_d": ones_r,
                "ones_ra_d": ones_r.astype(adt),
            }
        )
    return in_maps


def _assemble(results, queries):
    """results: 8 dicts with outT_d [EH, S] bf16 -> full [B, S, HID] f32
    with the queries residual added on host."""
    out = np.empty((B, S, HID), np.float32)
    for c in range(NCORES):
        b, hg = divmod(c, 2)
        out[b, :, hg * EH : (hg + 1) * EH] = (
            results[c]["outT_d"].astype(np.float32).T
            + queries[b, :, hg * EH : (hg + 1) * EH]
        )
    return out


def kernel(**inputs):
    from concourse.bass_utils import run_bass_kernel_spmd

    queries = np.asarray(inputs["queries"], np.float32)
    keys = np.asarray(inputs["keys"], np.float32)
    # fast path valid unless some key row is exactly zero-sum (padding)
    fast = not bool(np.any(keys.sum(axis=-1) == 0.0))
    has_bias = bool(
        np.any(np.asarray(inputs["bq"]))
        or np.any(np.asarray(inputs["bk"]))
        or np.any(np.asarray(inputs["bv"]))
    )
    nc = _build(fast=fast, has_bias=has_bias)
    in_maps = _host_prep(**inputs)
    res = run_bass_kernel_spmd(nc, in_maps, core_ids=list(range(NCORES)))
    kernel.last_results = res
    return _assemble(res.results, queries)
